# revision 8
# baseline (speedup 1.0000x reference)
"""TRN2 Bass kernel for nn_EnhanceLayer (GNN message passing with segment-max).

Strategy: shard edges by dst-node range across 8 cores (3125 nodes/core), so the
segment-max is core-local and no collective is needed. Per core:
  - host sorts edges by dst, pads each node's edge list to a multiple of W=4
    with duplicate edges (max is idempotent), packs 128-node tiles with a fixed
    group budget G_t (runtime-sized), and builds scan masks + one-hot extraction
    matrices.
  - device gathers h[src], h[dst] and node-level q*||q|| rows in bf16 via
    transposing dma_gather (features land on partitions), computes k/v/repe
    projections on the tensor engine, the score elementwise on DVE/ACT, a W=4
    grouped max, then a segmented running-max via tensor_tensor_scan, and
    extracts each node's last-group column with one-hot matmuls accumulated in
    PSUM (zero-degree nodes come out 0, matching the DGL convention).
Host reassembles the full [25000, 2, 4, 16] fp32 output.
"""

import sys

for _p in ("/opt/trn_rl_repo", "/root/.axon_site/_ro/trn_rl_repo"):
    if _p not in sys.path:
        sys.path.append(_p)

import numpy as np
import ml_dtypes

import concourse.mybir as mybir
import concourse.tile as tile
from concourse import bacc
from concourse import bass_utils

BF16 = ml_dtypes.bfloat16

N_NODES = 25000
N_EDGES = 400000
N_CORES = 8
NODES_PER_CORE = 3125
TILE_NODES = 128
N_TILES = 25  # 25*128 = 3200 node slots per core
W = 4  # level-1 reduce width
BLK = 512  # PSUM-friendly edge sub-block
NEG = -1e30


# ---------------------------------------------------------------- host prep --
def _prep_core(src, dst, core):
    lo = core * NODES_PER_CORE
    sel = np.where((dst >= lo) & (dst < lo + NODES_PER_CORE))[0]
    order = np.argsort(dst[sel], kind="stable")
    eidx = sel[order]
    dl = dst[eidx] - lo
    tiles = []
    for t in range(N_TILES):
        n0 = t * TILE_NODES
        m = (dl >= n0) & (dl < n0 + TILE_NODES)
        et, dlt = eidx[m], dl[m] - n0
        if et.size == 0:
            tiles.append((et, dlt, 0, None, None, None))
            continue
        uniq, starts = np.unique(dlt, return_index=True)
        counts = np.diff(np.append(starts, dlt.size))
        padded = ((counts + W - 1) // W) * W
        g = int(padded.sum() // W)
        tiles.append((et, dlt, g, uniq, starts, counts))
    return tiles


def _pack_tiles(core_tiles, G_t):
    """slot_edge [N_TILES, G_t*W] (global edge id per slot), mask [N_TILES, G_t]
    (NEG at each node's first group), selx [N_TILES, G_t, 128] one-hot mapping a
    node's last group -> its local column."""
    E_tile = G_t * W
    any_eid = 0
    for et, *_ in core_tiles:
        if et.size:
            any_eid = et[0]
            break
    slot_edge = np.full((N_TILES, E_tile), any_eid, dtype=np.int64)
    mask = np.zeros((N_TILES, G_t), dtype=np.float32)
    selx = np.zeros((N_TILES, G_t, TILE_NODES), dtype=np.float32)
    for t, (et, dlt, g, uniq, starts, counts) in enumerate(core_tiles):
        if et.size == 0:
            mask[t, 0] = NEG
            continue
        pos = 0
        for n_loc, s, cnt in zip(uniq, starts, counts):
            mask[t, pos // W] = NEG
            padded = int(-(-cnt // W)) * W
            ids = et[s : s + cnt]
            slot_edge[t, pos : pos + cnt] = ids
            slot_edge[t, pos + cnt : pos + padded] = ids[-1]
            pos += padded
            selx[t, pos // W - 1, n_loc] = 1.0
        if pos < E_tile:
            slot_edge[t, pos:] = et[-1]
            n_loc = dlt[-1]
            selx[t, :, n_loc] = 0.0
            selx[t, G_t - 1, n_loc] = 1.0
    return slot_edge, mask, selx


def _wrap_idx16(ids):
    """dma_gather index layout: idx i -> [i % 16, i // 16], replicated to 128
    partitions. ids: [n] -> [128, n // 16] int16."""
    n = ids.shape[0]
    w = ids.astype(np.int16).reshape(n // 16, 16).T  # [16, n//16]
    return np.tile(np.ascontiguousarray(w), (8, 1))  # [128, n//16]


def host_prep(src, dst):
    per_core = [_prep_core(src, dst, c) for c in range(N_CORES)]
    max_g = max((t[2] for ct in per_core for t in ct), default=1)
    G_t = max(128, ((max_g + 127) // 128) * 128)
    packed = [_pack_tiles(ct, G_t) for ct in per_core]
    return packed, G_t


def _dbl(w):
    """Stack two copies of a [64, 64] weight -> [128, 64] so a matmul whose rhs
    lives at partition base 64 can use an lhsT slice with the same base."""
    w = np.asarray(w, np.float32).astype(BF16)
    return np.concatenate([w, w], axis=0)


def _ones_consts():
    ones16 = np.zeros((128, 8), np.float32)
    for p in range(128):
        ones16[p, p // 16] = 1.0
    ones2 = ones16.T.copy()  # [8, 128]
    ident = np.eye(128, dtype=np.float32)
    return ones16.astype(BF16), ones2.astype(BF16), ident.astype(BF16)


def make_in_maps(h, e, W_Q, W_K, W_V, W_repe, src, dst, packed, G_t):
    h = np.asarray(h, np.float32).reshape(N_NODES, 128)
    e = np.asarray(e, np.float32).reshape(N_EDGES, 128)
    src = np.asarray(src).astype(np.int64)
    dst = np.asarray(dst).astype(np.int64)
    h_tab = h.astype(BF16)
    ones16, ones2, ident = _ones_consts()
    E_tile = G_t * W
    E_core = N_TILES * E_tile
    in_maps = []
    for c in range(N_CORES):
        lo = c * NODES_PER_CORE
        slot_edge, mask, selx = packed[c]
        ids = slot_edge.reshape(-1)  # [E_core]
        eT = np.ascontiguousarray(e[ids].astype(BF16).T)  # [128, E_core]
        srcI = _wrap_idx16(src[ids])
        dstG = _wrap_idx16(dst[ids])
        dstL = _wrap_idx16(dst[ids] - lo)
        hTl = np.zeros((128, N_TILES * 128), BF16)
        hTl[:, :NODES_PER_CORE] = h[lo : lo + NODES_PER_CORE].T.astype(BF16)
        maskb = np.ascontiguousarray(
            np.broadcast_to(mask.reshape(1, -1).astype(BF16), (128, N_TILES * G_t))
        )
        # selx blocks: [G_t rows (partitions), 128 node cols] per (tile, s)
        sx = selx.reshape(N_TILES, G_t // 128, 128, TILE_NODES)  # t, s, g128, n
        sxT = np.ascontiguousarray(sx.transpose(2, 0, 1, 3)).reshape(
            128, N_TILES * G_t
        )
        in_maps.append(
            {
                "h_tab": h_tab,
                "eT": eT,
                "srcI": srcI,
                "dstG": dstG,
                "dstL": dstL,
                "hT_local": hTl,
                "maskb": maskb.astype(BF16),
                "selx": sxT.astype(BF16),
                "w_q": _dbl(W_Q),
                "w_k": _dbl(W_K),
                "w_v": _dbl(W_V),
                "w_r": _dbl(W_repe),
                "ones16": ones16,
                "ones2": ones2,
                "ident": ident,
            }
        )
    return in_maps


# ------------------------------------------------------------ bass program --
def build_program(G_t, debug=False):
    E_tile = G_t * W
    n_blk = E_tile // BLK
    assert E_tile % BLK == 0
    E_core = N_TILES * E_tile
    G_core = N_TILES * G_t
    dt = mybir.dt
    op = mybir.AluOpType
    act = mybir.ActivationFunctionType

    nc = bacc.Bacc("TRN2", target_bir_lowering=False, debug=debug)

    h_tab = nc.dram_tensor("h_tab", [N_NODES, 128], dt.bfloat16, kind="ExternalInput")
    eT_d = nc.dram_tensor("eT", [128, E_core], dt.bfloat16, kind="ExternalInput")
    srcI_d = nc.dram_tensor("srcI", [128, E_core // 16], dt.int16, kind="ExternalInput")
    dstG_d = nc.dram_tensor("dstG", [128, E_core // 16], dt.int16, kind="ExternalInput")
    dstL_d = nc.dram_tensor("dstL", [128, E_core // 16], dt.int16, kind="ExternalInput")
    hTl_d = nc.dram_tensor("hT_local", [128, N_TILES * 128], dt.bfloat16, kind="ExternalInput")
    mask_d = nc.dram_tensor("maskb", [128, G_core], dt.bfloat16, kind="ExternalInput")
    selx_d = nc.dram_tensor("selx", [128, G_core], dt.bfloat16, kind="ExternalInput")
    wq_d = nc.dram_tensor("w_q", [128, 64], dt.bfloat16, kind="ExternalInput")
    wk_d = nc.dram_tensor("w_k", [128, 64], dt.bfloat16, kind="ExternalInput")
    wv_d = nc.dram_tensor("w_v", [128, 64], dt.bfloat16, kind="ExternalInput")
    wr_d = nc.dram_tensor("w_r", [128, 64], dt.bfloat16, kind="ExternalInput")
    ones16_d = nc.dram_tensor("ones16", [128, 8], dt.bfloat16, kind="ExternalInput")
    ones2_d = nc.dram_tensor("ones2", [8, 128], dt.bfloat16, kind="ExternalInput")
    ident_d = nc.dram_tensor("ident", [128, 128], dt.bfloat16, kind="ExternalInput")

    outT_d = nc.dram_tensor("outT", [128, N_TILES * 128], dt.float32, kind="ExternalOutput")
    qs_d = nc.dram_tensor("qs_nodes", [N_TILES * 128, 128], dt.bfloat16, kind="Internal")

    with tile.TileContext(nc) as tc:
        with (
            tc.tile_pool(name="const", bufs=1) as cpool,
            tc.tile_pool(name="gath", bufs=2) as gpool,
            tc.tile_pool(name="blk", bufs=2) as bpool,
            tc.tile_pool(name="tileb", bufs=2) as tpool,
            tc.tile_pool(name="small", bufs=2) as spool,
            tc.tile_pool(name="ps", bufs=1, space="PSUM") as ps,
            tc.tile_pool(name="ps2", bufs=2, space="PSUM") as ps2,
            tc.tile_pool(name="ps1", bufs=1, space="PSUM") as ps1,
        ):
            # ---- constants ----
            w_q = cpool.tile([128, 64], dt.bfloat16)
            w_k = cpool.tile([128, 64], dt.bfloat16)
            w_v = cpool.tile([128, 64], dt.bfloat16)
            w_r = cpool.tile([128, 64], dt.bfloat16)
            ones16 = cpool.tile([128, 8], dt.bfloat16)
            ones2 = cpool.tile([8, 128], dt.bfloat16)
            ident = cpool.tile([128, 128], dt.bfloat16)
            hTl0 = cpool.tile([64, N_TILES * 128], dt.bfloat16)
            hTl1 = cpool.tile([64, N_TILES * 128], dt.bfloat16)
            srcI = cpool.tile([128, E_core // 16], dt.int16)
            dstG = cpool.tile([128, E_core // 16], dt.int16)
            dstL = cpool.tile([128, E_core // 16], dt.int16)
            for sb, dr in (
                (w_q, wq_d), (w_k, wk_d), (w_v, wv_d), (w_r, wr_d),
                (ones16, ones16_d), (ones2, ones2_d), (ident, ident_d),
                (hTl0, hTl_d[0:64, :]), (hTl1, hTl_d[64:128, :]),
                (srcI, srcI_d), (dstG, dstG_d), (dstL, dstL_d),
            ):
                nc.sync.dma_start(out=sb[:], in_=dr[:])

            # ---- phase A: node-level qs = (h @ W_Q) * ||.||_{(n,h)} ----
            for wdw in range(N_TILES):
                q_ps = ps1.tile([128, 128], dt.float32, tag="qtr")
                for n, hTl_n in ((0, hTl0), (1, hTl1)):
                    nc.tensor.matmul(
                        out=q_ps[:, n * 64 : (n + 1) * 64],
                        lhsT=hTl_n[:, wdw * 128 : (wdw + 1) * 128],
                        rhs=w_q[0:64, :],
                        start=True,
                        stop=True,
                    )
                q_sb = spool.tile([128, 128], dt.bfloat16, tag="qsb")
                nc.scalar.activation(q_sb[:], q_ps[:], act.Copy)
                qsq = spool.tile([128, 128], dt.bfloat16, tag="qsq")
                nc.vector.tensor_tensor(out=qsq[:], in0=q_sb[:], in1=q_sb[:], op=op.mult)
                qsum = spool.tile([128, 8], dt.float32, tag="qsum")
                nc.vector.reduce_sum(
                    out=qsum[:],
                    in_=qsq[:].rearrange("p (g d) -> p g d", d=16),
                    axis=mybir.AxisListType.X,
                )
                qn = spool.tile([128, 8], dt.bfloat16, tag="qn")
                nc.scalar.activation(qn[:], qsum[:], act.Sqrt)
                qs_sb = spool.tile([128, 128], dt.bfloat16, tag="qs")
                nc.vector.tensor_tensor(
                    out=qs_sb[:].rearrange("p (g d) -> p g d", d=16),
                    in0=q_sb[:].rearrange("p (g d) -> p g d", d=16),
                    in1=qn[:].to_broadcast([128, 8, 16]),
                    op=op.mult,
                )
                nc.sync.dma_start(
                    out=qs_d[wdw * 128 : (wdw + 1) * 128, :], in_=qs_sb[:]
                )

            # ---- phase B: main edge pipeline, one 128-node tile at a time ----
            for t in range(N_TILES):
                ecols = slice(t * E_tile, (t + 1) * E_tile)
                gcols = slice(t * G_t, (t + 1) * G_t)
                icols = slice(t * (E_tile // 16), (t + 1) * (E_tile // 16))

                hs = gpool.tile([128, E_tile], dt.bfloat16, tag="hs")
                hd = gpool.tile([128, E_tile], dt.bfloat16, tag="hd")
                qe = gpool.tile([128, E_tile], dt.bfloat16, tag="qe")
                eT = gpool.tile([128, E_tile], dt.bfloat16, tag="eT")
                for out_t, table, idxs in (
                    (hs, h_tab, srcI),
                    (hd, h_tab, dstG),
                    (qe, qs_d, dstL),
                ):
                    nc.gpsimd.dma_gather(
                        out_ap=out_t[:].rearrange("p (o f) -> p o f", o=1),
                        in_ap=table[:],
                        idxs_ap=idxs[:, icols],
                        num_idxs=E_tile,
                        num_idxs_reg=E_tile,
                        elem_size=128,
                        transpose=True,
                        single_packet=False,
                    )
                nc.sync.dma_start(out=eT[:], in_=eT_d[:, ecols])
                maskb = tpool.tile([128, G_t], dt.bfloat16, tag="maskb")
                selx = tpool.tile([128, G_t], dt.bfloat16, tag="selx")
                nc.sync.dma_start(out=maskb[:], in_=mask_d[:, gcols])
                nc.sync.dma_start(out=selx[:], in_=selx_d[:, gcols])

                msgv = tpool.tile([128, E_tile], dt.bfloat16, tag="msgv")
                for b in range(n_blk):
                    bc = slice(b * BLK, (b + 1) * BLK)
                    d0 = bpool.tile([128, BLK], dt.bfloat16, tag="d0")
                    nc.vector.tensor_tensor(
                        out=d0[:], in0=hs[:, bc], in1=hd[:, bc], op=op.subtract
                    )
                    diff = bpool.tile([128, BLK], dt.bfloat16, tag="diff")
                    nc.scalar.activation(diff[:], d0[:], act.Relu)

                    k_ps = ps2.tile([128, BLK], dt.float32, tag="kps")
                    v_ps = ps.tile([128, BLK], dt.float32, tag="vps")
                    r_ps = ps.tile([128, BLK], dt.float32, tag="rps")
                    for n in range(2):
                        rows = slice(n * 64, (n + 1) * 64)
                        nc.tensor.matmul(out=k_ps[rows, :], lhsT=w_k[rows, :], rhs=diff[rows, :], start=True, stop=True)
                        nc.tensor.matmul(out=v_ps[rows, :], lhsT=w_v[rows, :], rhs=diff[rows, :], start=True, stop=True)
                        nc.tensor.matmul(out=r_ps[rows, :], lhsT=w_r[rows, :], rhs=eT[rows, bc], start=True, stop=True)

                    ksq = bpool.tile([128, BLK], dt.bfloat16, tag="ksq")
                    nc.scalar.activation(ksq[:], k_ps[:], act.Square)
                    ks_ps = ps.tile([128, BLK], dt.float32, tag="ksps")
                    nc.tensor.matmul(out=ks_ps[0:8, :], lhsT=ones16[:], rhs=ksq[:], start=True, stop=True)
                    knorm = bpool.tile([8, BLK], dt.bfloat16, tag="knorm")
                    nc.scalar.activation(knorm[:], ks_ps[0:8, :], act.Sqrt)
                    knb_ps = ps.tile([128, BLK], dt.float32, tag="knbps")
                    nc.tensor.matmul(out=knb_ps[:], lhsT=ones2[:], rhs=knorm[:], start=True, stop=True)

                    # elementwise score chain (PSUM operands read once each)
                    kq = bpool.tile([128, BLK], dt.bfloat16, tag="kq")
                    nc.vector.tensor_tensor(out=kq[:], in0=qe[:, bc], in1=k_ps[:], op=op.mult)
                    kqs = bpool.tile([128, BLK], dt.bfloat16, tag="kqs")
                    nc.vector.tensor_tensor(out=kqs[:], in0=kq[:], in1=knb_ps[:], op=op.mult)
                    sc = bpool.tile([128, BLK], dt.bfloat16, tag="sc")
                    nc.vector.tensor_tensor(out=sc[:], in0=kqs[:], in1=r_ps[:], op=op.add)
                    nc.vector.tensor_tensor(out=msgv[:, bc], in0=sc[:], in1=v_ps[:], op=op.mult)

                w4 = tpool.tile([128, G_t], dt.bfloat16, tag="w4")
                nc.vector.reduce_max(
                    out=w4[:],
                    in_=msgv[:].rearrange("p (g w) -> p g w", w=W),
                    axis=mybir.AxisListType.X,
                )
                scan = tpool.tile([128, G_t], dt.bfloat16, tag="scan")
                nc.vector.tensor_tensor_scan(
                    out=scan[:], data0=maskb[:], data1=w4[:],
                    initial=0.0, op0=op.add, op1=op.max,
                )

                o_ps = ps1.tile([128, 128], dt.float32, tag="ops")
                n_sel = G_t // 128
                for s in range(n_sel):
                    scols = slice(s * 128, (s + 1) * 128)
                    tr_ps = ps1.tile([128, 128], dt.bfloat16, tag="qtr")
                    nc.tensor.transpose(out=tr_ps[:], in_=scan[:, scols], identity=ident[:])
                    trT = spool.tile([128, 128], dt.bfloat16, tag="trT")
                    nc.vector.tensor_copy(out=trT[:], in_=tr_ps[:])
                    nc.tensor.matmul(
                        out=o_ps[:], lhsT=trT[:], rhs=selx[:, scols],
                        start=(s == 0), stop=(s == n_sel - 1),
                    )
                out_sb = spool.tile([128, 128], dt.float32, tag="outsb")
                nc.scalar.activation(out_sb[:], o_ps[:], act.Copy)
                nc.sync.dma_start(out=outT_d[:, t * 128 : (t + 1) * 128], in_=out_sb[:])

    nc.compile()
    return nc


# ------------------------------------------------------------------ driver --
_CACHE = {}


def run_on_cores(in_maps, G_t, trace=False):
    if G_t not in _CACHE:
        _CACHE[G_t] = build_program(G_t)
    nc = _CACHE[G_t]
    res = bass_utils.run_bass_kernel_spmd(
        nc, in_maps, core_ids=list(range(len(in_maps))), trace=trace
    )
    return res


def kernel(h, e, W_Q, W_K, W_V, W_repe, src, dst):
    src = np.asarray(src)
    dst = np.asarray(dst)
    packed, G_t = host_prep(src.astype(np.int64), dst.astype(np.int64))
    in_maps = make_in_maps(h, e, W_Q, W_K, W_V, W_repe, src, dst, packed, G_t)
    res = run_on_cores(in_maps, G_t)
    out = np.zeros((N_NODES, 128), np.float32)
    for c in range(N_CORES):
        lo = c * NODES_PER_CORE
        outT = res.results[c]["outT"]  # [128, 3200]
        out[lo : lo + NODES_PER_CORE] = outT.T[:NODES_PER_CORE]
    return out.reshape(N_NODES, 2, 4, 16)
